# revision 35
# baseline (speedup 1.0000x reference)
"""Trainium2 Bass kernel for nn_Attention_86431921864842.

Decode-style attention: B=16 batches, H=16 heads, Sq=16 new tokens,
4096-token KV cache, RoPE-extended 128-dim scores, fused QKV + output
projections.

Sharding: tensor-parallel over heads, 8 cores x 2 heads each.  Each core
receives the full x (bf16), its 2-head slice of w_qkv (transposed,
bf16), its 2-head column slice of w_o (transposed, bf16), and its heads'
K/rot/V caches as a single merged bf16 tensor per (head_local, batch):

  kv [32, 128, 6176] bf16 - cols 0:4096 = [cache_k; cache_pos_k_rot]^T
      (d2 = 128 on partitions, token on free); cols 4096:6176 = V cache
      tiled [p = token%128, tile, 64 dims + ones column] so the PV
      matmul also produces the softmax denominator.

The whole pipeline runs in single bf16 (inputs are f32; the 2e-2
correctness budget dwarfs bf16's ~1e-3 relative error), which halves
HBM traffic vs f32-grade storage and runs every matmul at the PE's
1 cycle/row bf16 rate.

Device per (b,h): S^T tiles via 1 bf16 matmul per 128-token tile
(k-chunk stationary, q [128,16] moving) -> exp (ACT, direct from PSUM,
bf16 out) -> PV accumulate (attn^T stationary, [V|1] moving) ->
per-query normalize -> o-proj partial.  PV for pair i is emitted after
S^T for pair i+1 (one-stage software pipeline) so the Tensor engine
never stalls on the exp.  Host sums the 8 partial o-proj outputs.
"""

import math
import os
import sys

import numpy as np

for _p in ("/opt/trn_rl_repo",):
    if _p not in sys.path and os.path.isdir(_p):
        sys.path.insert(0, _p)

B = 16
H = 16
SQ = 16
DM = 1024
DH = 64
SKV = 4096
ROPE_BASE = 10000.0
N_CORES = 8
H_PER_CORE = H // N_CORES  # 2
E_PER_CORE = H_PER_CORE * 3 * DH  # 384
D_PER_CORE = H_PER_CORE * DH  # 128
BS = B * SQ  # 256
N_KTILES = SKV // 128  # 32
VCOLS = N_KTILES * 65  # 2080
KVCOLS = SKV + VCOLS  # 6176
SCALE = 1.0 / math.sqrt(2 * DH)

_PROGRAM = None  # (nc, in_names, out_name)


def _build_program():
    import concourse.bass as bass
    import concourse.mybir as mybir
    import concourse.tile as tile
    from concourse import bacc

    f32 = mybir.dt.float32
    bf16 = mybir.dt.bfloat16
    Exp = mybir.ActivationFunctionType.Exp

    nc = bacc.Bacc(
        "TRN2",
        target_bir_lowering=False,
        debug=False,
        enable_asserts=False,
        num_devices=N_CORES,
    )

    x_d = nc.dram_tensor("xT", [128, 8, BS], bf16, kind="ExternalInput")
    wq_d = nc.dram_tensor("wqT", [128, 8, E_PER_CORE], bf16, kind="ExternalInput")
    wo_d = nc.dram_tensor("woT", [D_PER_CORE, DM], bf16, kind="ExternalInput")
    kv_d = nc.dram_tensor("kv", [2 * B, 128, KVCOLS], bf16, kind="ExternalInput")
    cos_d = nc.dram_tensor("cosN", [128, 32], f32, kind="ExternalInput")
    sin_d = nc.dram_tensor("sinN", [128, 32], f32, kind="ExternalInput")
    id_d = nc.dram_tensor("ident", [128, 128], f32, kind="ExternalInput")
    out_d = nc.dram_tensor("out", [2, 128, DM], bf16, kind="ExternalOutput")

    with tile.TileContext(nc) as tc:
        with (
            tc.tile_pool(name="const", bufs=1) as pc,
            tc.tile_pool(name="head", bufs=1) as ph,
            tc.tile_pool(name="rope", bufs=1) as pr,
            tc.tile_pool(name="kv", bufs=10) as pk,
            tc.tile_pool(name="exp", bufs=2) as pe,
            tc.tile_pool(name="small", bufs=2) as ps,
            tc.tile_pool(name="ps_s", bufs=2, space="PSUM") as pss,
            tc.tile_pool(name="ps_o", bufs=2, space="PSUM") as pso,
            tc.tile_pool(name="ps_m", bufs=2, space="PSUM") as psm,
        ):
            # ---- constants ----
            x_sb = pc.tile([128, 8, BS], bf16, tag="x")
            nc.sync.dma_start(x_sb[:], x_d[:])
            wq_sb = pc.tile([128, 8, E_PER_CORE], bf16, tag="wq")
            nc.sync.dma_start(wq_sb[:], wq_d[:])
            cos_sb = pc.tile([128, 32], f32, tag="cos")
            nc.sync.dma_start(cos_sb[:], cos_d[:])
            sin_sb = pc.tile([128, 32], f32, tag="sin")
            nc.sync.dma_start(sin_sb[:], sin_d[:])
            id_sb = pc.tile([128, 128], f32, tag="ident")
            nc.sync.dma_start(id_sb[:], id_d[:])
            wo_sb = pc.tile([128, DM], bf16, tag="wo")
            nc.sync.dma_start(wo_sb[:], wo_d[:])

            # ---- QKV projection: qkv_nat[bs, e_local] ----
            qkv_nat = ph.tile([128, 2, E_PER_CORE], f32, tag="qkv_nat")
            for j in range(2):
                psq = psm.tile([128, 512], f32, tag="misc", name=f"psq{j}")
                for dc in range(8):
                    nc.tensor.matmul(
                        psq[:, 0:E_PER_CORE],
                        lhsT=x_sb[:, dc, j * 128 : (j + 1) * 128],
                        rhs=wq_sb[:, dc, :],
                        start=(dc == 0),
                        stop=(dc == 7),
                    )
                nc.vector.tensor_copy(qkv_nat[:, j, :], psq[:, 0:E_PER_CORE])

            # ---- RoPE + transposes per local head ----
            cosb = cos_sb[:].unsqueeze(1).to_broadcast([128, 2, 32])
            sinb = sin_sb[:].unsqueeze(1).to_broadcast([128, 2, 32])
            q2T = []  # per head: [128, 256] f32 (d2, bs)
            q2B = []  # per head: [128, 256] bf16
            k2nT = []  # per head: [128, 256] f32
            k2nB = []  # per head: [128, 256] bf16
            vTh = []  # per head: [64, 256] f32 (dv, bs)
            for hl in range(2):
                base = hl * 3 * DH
                qs = qkv_nat[:, :, base : base + 64]
                ks = qkv_nat[:, :, base + 64 : base + 128]

                q2n = pr.tile([128, 2, 128], f32, tag="q2n")
                k2n = pr.tile([128, 2, 128], f32, tag="k2n")
                t1 = pr.tile([128, 2, 32], f32, tag="t1")
                t2 = pr.tile([128, 2, 32], f32, tag="t2")
                for src, dst in ((qs, q2n), (ks, k2n)):
                    x1 = src[:, :, 0:32]
                    x2 = src[:, :, 32:64]
                    nc.vector.tensor_copy(dst[:, :, 0:64], src)
                    nc.vector.tensor_mul(t1[:], x1, cosb)
                    nc.vector.tensor_mul(t2[:], x2, sinb)
                    nc.vector.tensor_sub(dst[:, :, 64:96], t1[:], t2[:])
                    nc.vector.tensor_mul(t1[:], x1, sinb)
                    nc.vector.tensor_mul(t2[:], x2, cosb)
                    nc.vector.tensor_add(dst[:, :, 96:128], t1[:], t2[:])

                q2T_h = ph.tile([128, BS], f32, tag=f"q2T_{hl}")
                k2nT_h = ph.tile([128, BS], f32, tag=f"k2nT_{hl}")
                vT_h = ph.tile([64, BS], f32, tag=f"vT_{hl}")
                for j in range(2):
                    pt = psm.tile([128, 512], f32, tag="misc")
                    nc.tensor.transpose(pt[:, 0:128], q2n[:, j, :], id_sb[:])
                    nc.vector.tensor_copy(q2T_h[:, j * 128 : (j + 1) * 128], pt[:, 0:128])
                    pt2 = psm.tile([128, 512], f32, tag="misc")
                    nc.tensor.transpose(pt2[:, 0:128], k2n[:, j, :], id_sb[:])
                    nc.vector.tensor_copy(
                        k2nT_h[:, j * 128 : (j + 1) * 128], pt2[:, 0:128]
                    )
                    pt3 = psm.tile([128, 512], f32, tag="misc")
                    nc.tensor.transpose(
                        pt3[0:64, 0:128],
                        qkv_nat[:, j, base + 128 : base + 192],
                        id_sb[:],
                    )
                    nc.vector.tensor_copy(vT_h[:, j * 128 : (j + 1) * 128], pt3[0:64, 0:128])

                q2b_h = ph.tile([128, BS], bf16, tag=f"q2b_{hl}")
                nc.vector.tensor_copy(q2b_h[:], q2T_h[:])
                k2nb_h = ph.tile([128, BS], bf16, tag=f"k2nb_{hl}")
                nc.vector.tensor_copy(k2nb_h[:], k2nT_h[:])

                q2T.append(q2T_h)
                q2B.append(q2b_h)
                k2nT.append(k2nT_h)
                k2nB.append(k2nb_h)
                vTh.append(vT_h)

            # ---- new-token V rows, pre-transposed to [s, (hl,b), 65] ----
            # (transposes are emitted inside the main loop, spread across
            # the first iterations, to keep the serial PE<->DVE ping-pong
            # off the startup critical path)
            vn_all = ph.tile([16, 2, B, 65], bf16, tag="vn_all")
            nc.vector.memset(vn_all[:, :, :, 64:65], 1.0)

            def emit_vn(hl, b):
                pvn = psm.tile([128, 512], f32, tag="misc")
                nc.tensor.transpose(
                    pvn[0:16, 0:64],
                    vTh[hl][:, b * 16 : (b + 1) * 16],
                    id_sb[0:64, 0:64],
                )
                nc.vector.tensor_copy(vn_all[:, hl, b, 0:64], pvn[0:16, 0:64])

            # val_sb[s, b, hl, dv] : normalized attention output (natural)
            val_sb = ph.tile([16, B, 2, 64], f32, tag="val_sb")

            # ---- main loop over (head_local, batch), PV pipelined 1 back ----
            def emit_pv(state):
                hl, b, expA, expB, kv_t = state
                ps_o = pso.tile([16, 65], f32, tag="o")
                for i in range(N_KTILES):
                    e = expA if i < 16 else expB
                    c = i if i < 16 else i - 16
                    nc.tensor.matmul(
                        ps_o[:],
                        lhsT=e[:, c * 16 : (c + 1) * 16],
                        rhs=kv_t[:, SKV + i * 65 : SKV + (i + 1) * 65],
                        start=(i == 0),
                        stop=False,
                    )
                nc.tensor.matmul(
                    ps_o[:],
                    lhsT=expB[0:16, 256:272],
                    rhs=vn_all[:, hl, b, :],
                    start=False,
                    stop=True,
                )
                rec = ps.tile([16, 1], f32, tag="rec")
                nc.vector.reciprocal(rec[:], ps_o[:, 64:65])
                nc.vector.tensor_mul(
                    val_sb[:, b, hl, :],
                    ps_o[:, 0:64],
                    rec[:, 0:1].to_broadcast([16, 64]),
                )

            # epilogue piece for one bs-chunk (8 batches x both heads)
            valT = ph.tile([128, 2, 128], bf16, tag="valT")
            out_sb = ph.tile([128, 2, DM], bf16, tag="out_sb")

            def emit_chunk_epilogue(j):
                pvt = psm.tile([128, 512], f32, tag="misc", name=f"pvt{j}")
                for bb in range(8):
                    b = j * 8 + bb
                    nc.tensor.transpose(
                        pvt[:, bb * 16 : (bb + 1) * 16],
                        val_sb[:, b, :, :],
                        id_sb[0:16, 0:16],
                    )
                nc.vector.tensor_copy(valT[:, j, :], pvt[:, 0:128])
                for h2 in range(2):
                    po = psm.tile([128, 512], f32, tag="misc", name=f"po{j}{h2}")
                    nc.tensor.matmul(
                        po[:],
                        lhsT=valT[:, j, :],
                        rhs=wo_sb[:, h2 * 512 : (h2 + 1) * 512],
                        start=True,
                        stop=True,
                    )
                    nc.vector.tensor_copy(
                        out_sb[:, j, h2 * 512 : (h2 + 1) * 512], po[:]
                    )
                # out goes on the gpsimd queue: the sync queue carries the kv
                # stream (a not-yet-ready out at the ring head would stall all
                # later kv transfers), and on the scalar queue the out's wait
                # head-of-line blocks the ACT sequencer, pushing every later
                # exp a full pair late and breaking the PV pipeline.
                nc.gpsimd.dma_start(out_d[j], out_sb[:, j, :])

            def emit_half_epilogue(j, half):
                # bs-half of chunk j: 4 transposes -> half o-proj -> half out
                # DMA; used at the very end so the first half's out transfer
                # overlaps the second half's compute
                pvt = psm.tile([128, 512], f32, tag="misc", name=f"pvh{j}{half}")
                for bb in range(4):
                    b = j * 8 + half * 4 + bb
                    nc.tensor.transpose(
                        pvt[:, bb * 16 : (bb + 1) * 16],
                        val_sb[:, b, :, :],
                        id_sb[0:16, 0:16],
                    )
                lo = half * 64
                nc.vector.tensor_copy(valT[:, j, lo : lo + 64], pvt[:, 0:64])
                for h2 in range(2):
                    po = psm.tile([128, 512], f32, tag="misc", name=f"poh{j}{half}{h2}")
                    nc.tensor.matmul(
                        po[0:64, :],
                        lhsT=valT[:, j, lo : lo + 64],
                        rhs=wo_sb[:, h2 * 512 : (h2 + 1) * 512],
                        start=True,
                        stop=True,
                    )
                    nc.vector.tensor_copy(
                        out_sb[lo : lo + 64, j, h2 * 512 : (h2 + 1) * 512],
                        po[0:64, :],
                    )
                nc.gpsimd.dma_start(
                    out_d[j, lo : lo + 64, :], out_sb[lo : lo + 64, j, :]
                )

            pending = None
            n_pv_done = 0
            for b in range(B):
                for hl in range(2):
                    bh = hl * B + b
                    kv_t = pk.tile([128, KVCOLS], bf16, tag="kv")
                    nc.sync.dma_start(kv_t[:], kv_d[bh])

                    if hl == 0:
                        # new-token V rows for this batch, both heads (needed
                        # by this pair's PV one iteration later)
                        emit_vn(0, b)
                        emit_vn(1, b)

                    # new-token scores (bf16, tiny; independent of the kv
                    # DMA, so it gives PE work at the bh boundary)
                    psn = psm.tile([16, 16], f32, tag="misc", name=f"psn{bh}")
                    nc.tensor.matmul(
                        psn[:],
                        lhsT=k2nB[hl][:, b * 16 : (b + 1) * 16],
                        rhs=q2B[hl][:, b * 16 : (b + 1) * 16],
                        start=True,
                        stop=True,
                    )

                    # S^T: one bf16 matmul per 128-token tile.  Scores and
                    # exp are split in half (separate PSUM + expT tiles) so
                    # the first exp is emitted - and its PE wait resolves -
                    # mid-scores; PV's first 16 chunks then depend only on
                    # expA, hiding the exp+sem latency inside the pair.
                    qb = q2B[hl][:, b * 16 : (b + 1) * 16]  # [128, 16] bf16
                    psA = pss.tile([128, 256], f32, tag="sTA")
                    psB = pss.tile([128, 256], f32, tag="sTB")
                    for i in range(16):
                        nc.tensor.matmul(
                            psA[:, i * 16 : (i + 1) * 16],
                            lhsT=kv_t[:, i * 128 : (i + 1) * 128],
                            rhs=qb,
                            start=True,
                            stop=True,
                        )
                    expA = pe.tile([128, 256], bf16, tag="expA")
                    nc.scalar.activation(expA[:], psA[:], Exp, scale=SCALE)
                    for i in range(16, N_KTILES):
                        nc.tensor.matmul(
                            psB[:, (i - 16) * 16 : (i - 15) * 16],
                            lhsT=kv_t[:, i * 128 : (i + 1) * 128],
                            rhs=qb,
                            start=True,
                            stop=True,
                        )
                    expB = pe.tile([128, 272], bf16, tag="expB")
                    nc.scalar.activation(expB[:, 0:256], psB[:], Exp, scale=SCALE)
                    nc.scalar.activation(
                        expB[0:16, 256:272], psn[:], Exp, scale=SCALE
                    )

                    if pending is not None:
                        emit_pv(pending)
                        n_pv_done += 1
                        if n_pv_done == 17:
                            # batches 0..7 (both heads) fully normalized:
                            # run the first output-chunk epilogue now
                            emit_chunk_epilogue(0)
                    pending = (hl, b, expA, expB, kv_t)
            emit_pv(pending)
            emit_half_epilogue(1, 0)
            emit_half_epilogue(1, 1)

    nc.compile()
    in_names = ["xT", "wqT", "woT", "kv", "cosN", "sinN", "ident"]
    return nc, in_names, "out"


def _get_program():
    global _PROGRAM
    if _PROGRAM is None:
        _PROGRAM = _build_program()
    return _PROGRAM


def _prep_inputs(x, w_qkv, w_o, cache_k, cache_v, cache_pos_k_rot):
    """Host-side sharding + layout prep. Returns list of per-core in_maps."""
    import ml_dtypes

    f32 = np.float32
    bf16 = ml_dtypes.bfloat16
    x = np.ascontiguousarray(x, dtype=f32)
    w_qkv = np.ascontiguousarray(w_qkv, dtype=f32)
    w_o = np.ascontiguousarray(w_o, dtype=f32)

    xT = np.ascontiguousarray(x.reshape(BS, DM).T).astype(bf16)
    # pre-tile to [p=128, dc=8, bs] so the const DMA is contiguous per row
    xT = np.ascontiguousarray(xT.reshape(8, 128, BS).transpose(1, 0, 2))

    wqkvT = np.ascontiguousarray(w_qkv.T).astype(bf16)  # [DM, 3*DM]

    # merged K2^T | V-tiles staging, per core: [2, B, 128, KVCOLS] bf16
    kv = np.empty((N_CORES, 2, B, 128, KVCOLS), dtype=bf16)
    kv[:, :, :, 0:64, 0:SKV] = cache_k.reshape(B, N_CORES, 2, SKV, DH).transpose(
        1, 2, 0, 4, 3
    )
    kv[:, :, :, 64:128, 0:SKV] = cache_pos_k_rot.reshape(
        B, N_CORES, 2, SKV, DH
    ).transpose(1, 2, 0, 4, 3)
    vpart = kv[:, :, :, :, SKV:].reshape(N_CORES, 2, B, 128, N_KTILES, 65)
    vpart[..., 0:64] = cache_v.reshape(B, N_CORES, 2, N_KTILES, 128, DH).transpose(
        1, 2, 0, 4, 3, 5
    )
    vpart[..., 64] = 1.0

    # RoPE tables, f32 math mirroring the reference
    j2 = np.arange(0, DH, 2, dtype=f32)
    inv_freq = (1.0 / (ROPE_BASE ** (j2 / f32(DH)))).astype(f32)
    pos = (SKV + np.arange(SQ)).astype(f32)
    ang = pos[:, None] * inv_freq[None, :]  # [16, 32]
    cosN = np.tile(np.cos(ang).astype(f32), (8, 1))  # [128, 32]
    sinN = np.tile(np.sin(ang).astype(f32), (8, 1))

    ident = np.eye(128, dtype=f32)

    in_maps = []
    for c in range(N_CORES):
        wq_c = wqkvT[:, c * E_PER_CORE : (c + 1) * E_PER_CORE]
        wq_c = np.ascontiguousarray(
            wq_c.reshape(8, 128, E_PER_CORE).transpose(1, 0, 2)
        )
        in_maps.append(
            {
                "xT": xT,
                "wqT": wq_c,
                "woT": np.ascontiguousarray(
                    w_o[:, c * D_PER_CORE : (c + 1) * D_PER_CORE].T
                ).astype(bf16),
                "kv": kv[c].reshape(2 * B, 128, KVCOLS),
                "cosN": cosN,
                "sinN": sinN,
                "ident": ident,
            }
        )
    return in_maps


def _run(in_maps, trace=False, trace_kwargs=None):
    from concourse import bass_utils

    nc, in_names, out_name = _get_program()
    kwargs = {}
    if trace:
        kwargs["trace"] = True
        if trace_kwargs:
            kwargs.update(trace_kwargs)
    res = bass_utils.run_bass_kernel_spmd(
        nc, in_maps, core_ids=list(range(N_CORES)), **kwargs
    )
    return res


def kernel(x, w_qkv, w_o, cache_k, cache_v, cache_pos_k_rot, mask=None, **_ignored):
    """Full-input entry point: shards internally across 8 NeuronCores."""
    in_maps = _prep_inputs(x, w_qkv, w_o, cache_k, cache_v, cache_pos_k_rot)
    res = _run(in_maps)
    out = np.zeros((BS, DM), dtype=np.float32)
    for c in range(N_CORES):
        out += res.results[c]["out"].reshape(BS, DM)
    return out.reshape(B, SQ, DM)


# revision 37
# speedup vs baseline: 1.0153x; 1.0153x over previous
"""Trainium2 Bass kernel for nn_Attention_86431921864842.

Decode-style attention: B=16 batches, H=16 heads, Sq=16 new tokens,
4096-token KV cache, RoPE-extended 128-dim scores, fused QKV + output
projections.

Sharding: tensor-parallel over heads, 8 cores x 2 heads each.  Each core
receives the full x (bf16), its 2-head slice of w_qkv (transposed,
bf16), its 2-head column slice of w_o (transposed, bf16), and its heads'
K/rot/V caches as a single merged bf16 tensor per (head_local, batch):

  kv [32, 128, 6176] bf16 - cols 0:4096 = [cache_k; cache_pos_k_rot]^T
      (d2 = 128 on partitions, token on free); cols 4096:6176 = V cache
      tiled [p = token%128, tile, 64 dims + ones column] so the PV
      matmul also produces the softmax denominator.

The whole pipeline runs in single bf16 (inputs are f32; the 2e-2
correctness budget dwarfs bf16's ~1e-3 relative error), which halves
HBM traffic vs f32-grade storage and runs every matmul at the PE's
1 cycle/row bf16 rate.

Device per (b,h): S^T tiles via 1 bf16 matmul per 128-token tile
(k-chunk stationary, q [128,16] moving) -> exp (ACT, direct from PSUM,
bf16 out) -> PV accumulate (attn^T stationary, [V|1] moving) ->
per-query normalize -> o-proj partial.  PV for pair i is emitted after
S^T for pair i+1 (one-stage software pipeline) so the Tensor engine
never stalls on the exp.  Host sums the 8 partial o-proj outputs.
"""

import math
import os
import sys

import numpy as np

for _p in ("/opt/trn_rl_repo",):
    if _p not in sys.path and os.path.isdir(_p):
        sys.path.insert(0, _p)

B = 16
H = 16
SQ = 16
DM = 1024
DH = 64
SKV = 4096
ROPE_BASE = 10000.0
N_CORES = 8
H_PER_CORE = H // N_CORES  # 2
E_PER_CORE = H_PER_CORE * 3 * DH  # 384
D_PER_CORE = H_PER_CORE * DH  # 128
BS = B * SQ  # 256
N_KTILES = SKV // 128  # 32
VCOLS = N_KTILES * 65  # 2080
KVCOLS = SKV + VCOLS  # 6176
SCALE = 1.0 / math.sqrt(2 * DH)

_PROGRAM = None  # (nc, in_names, out_name)


def _build_program():
    import concourse.bass as bass
    import concourse.mybir as mybir
    import concourse.tile as tile
    from concourse import bacc

    f32 = mybir.dt.float32
    bf16 = mybir.dt.bfloat16
    Exp = mybir.ActivationFunctionType.Exp

    nc = bacc.Bacc(
        "TRN2",
        target_bir_lowering=False,
        debug=False,
        enable_asserts=False,
        num_devices=N_CORES,
    )

    x_d = nc.dram_tensor("xT", [128, 8, BS], bf16, kind="ExternalInput")
    wq_d = nc.dram_tensor("wqT", [128, 8, E_PER_CORE], bf16, kind="ExternalInput")
    wo_d = nc.dram_tensor("woT", [D_PER_CORE, DM], bf16, kind="ExternalInput")
    kv_d = nc.dram_tensor("kv", [2 * B, 128, KVCOLS], bf16, kind="ExternalInput")
    cos_d = nc.dram_tensor("cosN", [128, 32], f32, kind="ExternalInput")
    sin_d = nc.dram_tensor("sinN", [128, 32], f32, kind="ExternalInput")
    id_d = nc.dram_tensor("ident", [128, 128], f32, kind="ExternalInput")
    out_d = nc.dram_tensor("out", [2, 128, DM], bf16, kind="ExternalOutput")

    with tile.TileContext(nc) as tc:
        with (
            tc.tile_pool(name="const", bufs=1) as pc,
            tc.tile_pool(name="head", bufs=1) as ph,
            tc.tile_pool(name="rope", bufs=1) as pr,
            tc.tile_pool(name="kv", bufs=10) as pk,
            tc.tile_pool(name="exp", bufs=2) as pe,
            tc.tile_pool(name="small", bufs=2) as ps,
            tc.tile_pool(name="ps_s", bufs=2, space="PSUM") as pss,
            tc.tile_pool(name="ps_o", bufs=2, space="PSUM") as pso,
            tc.tile_pool(name="ps_m", bufs=2, space="PSUM") as psm,
        ):
            # ---- constants ----
            x_sb = pc.tile([128, 8, BS], bf16, tag="x")
            nc.sync.dma_start(x_sb[:], x_d[:])
            wq_sb = pc.tile([128, 8, E_PER_CORE], bf16, tag="wq")
            nc.sync.dma_start(wq_sb[:], wq_d[:])
            cos_sb = pc.tile([128, 32], f32, tag="cos")
            nc.sync.dma_start(cos_sb[:], cos_d[:])
            sin_sb = pc.tile([128, 32], f32, tag="sin")
            nc.sync.dma_start(sin_sb[:], sin_d[:])
            id_sb = pc.tile([128, 128], f32, tag="ident")
            nc.sync.dma_start(id_sb[:], id_d[:])
            wo_sb = pc.tile([128, DM], bf16, tag="wo")
            # wo is first needed by the mid-kernel epilogue; fetch it on the
            # gpsimd queue so the sync queue reaches the kv stream sooner
            nc.gpsimd.dma_start(wo_sb[:], wo_d[:])

            # ---- QKV projection: qkv_nat[bs, e_local] ----
            qkv_nat = ph.tile([128, 2, E_PER_CORE], f32, tag="qkv_nat")
            for j in range(2):
                psq = psm.tile([128, 512], f32, tag="misc", name=f"psq{j}")
                for dc in range(8):
                    nc.tensor.matmul(
                        psq[:, 0:E_PER_CORE],
                        lhsT=x_sb[:, dc, j * 128 : (j + 1) * 128],
                        rhs=wq_sb[:, dc, :],
                        start=(dc == 0),
                        stop=(dc == 7),
                    )
                nc.vector.tensor_copy(qkv_nat[:, j, :], psq[:, 0:E_PER_CORE])

            # ---- RoPE + transposes per local head ----
            cosb = cos_sb[:].unsqueeze(1).to_broadcast([128, 2, 32])
            sinb = sin_sb[:].unsqueeze(1).to_broadcast([128, 2, 32])
            q2T = []  # per head: [128, 256] f32 (d2, bs)
            q2B = []  # per head: [128, 256] bf16
            k2nT = []  # per head: [128, 256] f32
            k2nB = []  # per head: [128, 256] bf16
            vTh = []  # per head: [64, 256] f32 (dv, bs)
            for hl in range(2):
                base = hl * 3 * DH
                qs = qkv_nat[:, :, base : base + 64]
                ks = qkv_nat[:, :, base + 64 : base + 128]

                q2n = pr.tile([128, 2, 128], f32, tag="q2n")
                k2n = pr.tile([128, 2, 128], f32, tag="k2n")
                t1 = pr.tile([128, 2, 32], f32, tag="t1")
                t2 = pr.tile([128, 2, 32], f32, tag="t2")
                for src, dst in ((qs, q2n), (ks, k2n)):
                    x1 = src[:, :, 0:32]
                    x2 = src[:, :, 32:64]
                    nc.vector.tensor_copy(dst[:, :, 0:64], src)
                    nc.vector.tensor_mul(t1[:], x1, cosb)
                    nc.vector.tensor_mul(t2[:], x2, sinb)
                    nc.vector.tensor_sub(dst[:, :, 64:96], t1[:], t2[:])
                    nc.vector.tensor_mul(t1[:], x1, sinb)
                    nc.vector.tensor_mul(t2[:], x2, cosb)
                    nc.vector.tensor_add(dst[:, :, 96:128], t1[:], t2[:])

                q2T_h = ph.tile([128, BS], f32, tag=f"q2T_{hl}")
                k2nT_h = ph.tile([128, BS], f32, tag=f"k2nT_{hl}")
                vT_h = ph.tile([64, BS], f32, tag=f"vT_{hl}")
                for j in range(2):
                    pt = psm.tile([128, 512], f32, tag="misc")
                    nc.tensor.transpose(pt[:, 0:128], q2n[:, j, :], id_sb[:])
                    nc.vector.tensor_copy(q2T_h[:, j * 128 : (j + 1) * 128], pt[:, 0:128])
                    pt2 = psm.tile([128, 512], f32, tag="misc")
                    nc.tensor.transpose(pt2[:, 0:128], k2n[:, j, :], id_sb[:])
                    nc.vector.tensor_copy(
                        k2nT_h[:, j * 128 : (j + 1) * 128], pt2[:, 0:128]
                    )
                    pt3 = psm.tile([128, 512], f32, tag="misc")
                    nc.tensor.transpose(
                        pt3[0:64, 0:128],
                        qkv_nat[:, j, base + 128 : base + 192],
                        id_sb[:],
                    )
                    nc.vector.tensor_copy(vT_h[:, j * 128 : (j + 1) * 128], pt3[0:64, 0:128])

                q2b_h = ph.tile([128, BS], bf16, tag=f"q2b_{hl}")
                nc.vector.tensor_copy(q2b_h[:], q2T_h[:])
                k2nb_h = ph.tile([128, BS], bf16, tag=f"k2nb_{hl}")
                nc.vector.tensor_copy(k2nb_h[:], k2nT_h[:])

                q2T.append(q2T_h)
                q2B.append(q2b_h)
                k2nT.append(k2nT_h)
                k2nB.append(k2nb_h)
                vTh.append(vT_h)

            # ---- new-token V rows, pre-transposed to [s, (hl,b), 65] ----
            # (transposes are emitted inside the main loop, spread across
            # the first iterations, to keep the serial PE<->DVE ping-pong
            # off the startup critical path)
            vn_all = ph.tile([16, 2, B, 65], bf16, tag="vn_all")
            nc.vector.memset(vn_all[:, :, :, 64:65], 1.0)

            def emit_vn(hl, b):
                pvn = psm.tile([128, 512], f32, tag="misc")
                nc.tensor.transpose(
                    pvn[0:16, 0:64],
                    vTh[hl][:, b * 16 : (b + 1) * 16],
                    id_sb[0:64, 0:64],
                )
                nc.vector.tensor_copy(vn_all[:, hl, b, 0:64], pvn[0:16, 0:64])

            # val_sb[s, b, hl, dv] : normalized attention output (natural)
            val_sb = ph.tile([16, B, 2, 64], f32, tag="val_sb")

            # ---- main loop over (head_local, batch), PV pipelined 1 back ----
            def emit_pv(state):
                hl, b, expA, expB, kv_t = state
                ps_o = pso.tile([16, 65], f32, tag="o")
                for i in range(N_KTILES):
                    e = expA if i < 16 else expB
                    c = i if i < 16 else i - 16
                    nc.tensor.matmul(
                        ps_o[:],
                        lhsT=e[:, c * 16 : (c + 1) * 16],
                        rhs=kv_t[:, SKV + i * 65 : SKV + (i + 1) * 65],
                        start=(i == 0),
                        stop=False,
                    )
                nc.tensor.matmul(
                    ps_o[:],
                    lhsT=expB[0:16, 256:272],
                    rhs=vn_all[:, hl, b, :],
                    start=False,
                    stop=True,
                )
                rec = ps.tile([16, 1], f32, tag="rec")
                nc.vector.reciprocal(rec[:], ps_o[:, 64:65])
                nc.vector.tensor_mul(
                    val_sb[:, b, hl, :],
                    ps_o[:, 0:64],
                    rec[:, 0:1].to_broadcast([16, 64]),
                )

            # epilogue piece for one bs-chunk (8 batches x both heads)
            valT = ph.tile([128, 2, 128], bf16, tag="valT")
            out_sb = ph.tile([128, 2, DM], bf16, tag="out_sb")

            def emit_chunk_epilogue(j):
                pvt = psm.tile([128, 512], f32, tag="misc", name=f"pvt{j}")
                for bb in range(8):
                    b = j * 8 + bb
                    nc.tensor.transpose(
                        pvt[:, bb * 16 : (bb + 1) * 16],
                        val_sb[:, b, :, :],
                        id_sb[0:16, 0:16],
                    )
                nc.vector.tensor_copy(valT[:, j, :], pvt[:, 0:128])
                for h2 in range(2):
                    po = psm.tile([128, 512], f32, tag="misc", name=f"po{j}{h2}")
                    nc.tensor.matmul(
                        po[:],
                        lhsT=valT[:, j, :],
                        rhs=wo_sb[:, h2 * 512 : (h2 + 1) * 512],
                        start=True,
                        stop=True,
                    )
                    nc.vector.tensor_copy(
                        out_sb[:, j, h2 * 512 : (h2 + 1) * 512], po[:]
                    )
                # out goes on the gpsimd queue: the sync queue carries the kv
                # stream (a not-yet-ready out at the ring head would stall all
                # later kv transfers), and on the scalar queue the out's wait
                # head-of-line blocks the ACT sequencer, pushing every later
                # exp a full pair late and breaking the PV pipeline.
                nc.gpsimd.dma_start(out_d[j], out_sb[:, j, :])

            pending = None
            n_pv_done = 0
            for b in range(B):
                for hl in range(2):
                    bh = hl * B + b
                    kv_t = pk.tile([128, KVCOLS], bf16, tag="kv")
                    nc.sync.dma_start(kv_t[:], kv_d[bh])

                    if hl == 0:
                        # new-token V rows for this batch, both heads (needed
                        # by this pair's PV one iteration later)
                        emit_vn(0, b)
                        emit_vn(1, b)

                    # new-token scores (bf16, tiny; independent of the kv
                    # DMA, so it gives PE work at the bh boundary)
                    psn = psm.tile([16, 16], f32, tag="misc", name=f"psn{bh}")
                    nc.tensor.matmul(
                        psn[:],
                        lhsT=k2nB[hl][:, b * 16 : (b + 1) * 16],
                        rhs=q2B[hl][:, b * 16 : (b + 1) * 16],
                        start=True,
                        stop=True,
                    )

                    # S^T: one bf16 matmul per 128-token tile.  Scores and
                    # exp are split in half (separate PSUM + expT tiles) so
                    # the first exp is emitted - and its PE wait resolves -
                    # mid-scores; PV's first 16 chunks then depend only on
                    # expA, hiding the exp+sem latency inside the pair.
                    qb = q2B[hl][:, b * 16 : (b + 1) * 16]  # [128, 16] bf16
                    psA = pss.tile([128, 256], f32, tag="sTA")
                    psB = pss.tile([128, 256], f32, tag="sTB")
                    for i in range(16):
                        nc.tensor.matmul(
                            psA[:, i * 16 : (i + 1) * 16],
                            lhsT=kv_t[:, i * 128 : (i + 1) * 128],
                            rhs=qb,
                            start=True,
                            stop=True,
                        )
                    expA = pe.tile([128, 256], bf16, tag="expA")
                    nc.scalar.activation(expA[:], psA[:], Exp, scale=SCALE)
                    for i in range(16, N_KTILES):
                        nc.tensor.matmul(
                            psB[:, (i - 16) * 16 : (i - 15) * 16],
                            lhsT=kv_t[:, i * 128 : (i + 1) * 128],
                            rhs=qb,
                            start=True,
                            stop=True,
                        )
                    expB = pe.tile([128, 272], bf16, tag="expB")
                    nc.scalar.activation(expB[:, 0:256], psB[:], Exp, scale=SCALE)
                    nc.scalar.activation(
                        expB[0:16, 256:272], psn[:], Exp, scale=SCALE
                    )

                    if pending is not None:
                        emit_pv(pending)
                        n_pv_done += 1
                        if n_pv_done == 17:
                            # batches 0..7 (both heads) fully normalized:
                            # run the first output-chunk epilogue now
                            emit_chunk_epilogue(0)
                    pending = (hl, b, expA, expB, kv_t)
            emit_pv(pending)
            emit_chunk_epilogue(1)

    nc.compile()
    in_names = ["xT", "wqT", "woT", "kv", "cosN", "sinN", "ident"]
    return nc, in_names, "out"


def _get_program():
    global _PROGRAM
    if _PROGRAM is None:
        _PROGRAM = _build_program()
    return _PROGRAM


def _prep_inputs(x, w_qkv, w_o, cache_k, cache_v, cache_pos_k_rot):
    """Host-side sharding + layout prep. Returns list of per-core in_maps."""
    import ml_dtypes

    f32 = np.float32
    bf16 = ml_dtypes.bfloat16
    x = np.ascontiguousarray(x, dtype=f32)
    w_qkv = np.ascontiguousarray(w_qkv, dtype=f32)
    w_o = np.ascontiguousarray(w_o, dtype=f32)

    xT = np.ascontiguousarray(x.reshape(BS, DM).T).astype(bf16)
    # pre-tile to [p=128, dc=8, bs] so the const DMA is contiguous per row
    xT = np.ascontiguousarray(xT.reshape(8, 128, BS).transpose(1, 0, 2))

    wqkvT = np.ascontiguousarray(w_qkv.T).astype(bf16)  # [DM, 3*DM]

    # merged K2^T | V-tiles staging, per core: [2, B, 128, KVCOLS] bf16
    kv = np.empty((N_CORES, 2, B, 128, KVCOLS), dtype=bf16)
    kv[:, :, :, 0:64, 0:SKV] = cache_k.reshape(B, N_CORES, 2, SKV, DH).transpose(
        1, 2, 0, 4, 3
    )
    kv[:, :, :, 64:128, 0:SKV] = cache_pos_k_rot.reshape(
        B, N_CORES, 2, SKV, DH
    ).transpose(1, 2, 0, 4, 3)
    vpart = kv[:, :, :, :, SKV:].reshape(N_CORES, 2, B, 128, N_KTILES, 65)
    vpart[..., 0:64] = cache_v.reshape(B, N_CORES, 2, N_KTILES, 128, DH).transpose(
        1, 2, 0, 4, 3, 5
    )
    vpart[..., 64] = 1.0

    # RoPE tables, f32 math mirroring the reference
    j2 = np.arange(0, DH, 2, dtype=f32)
    inv_freq = (1.0 / (ROPE_BASE ** (j2 / f32(DH)))).astype(f32)
    pos = (SKV + np.arange(SQ)).astype(f32)
    ang = pos[:, None] * inv_freq[None, :]  # [16, 32]
    cosN = np.tile(np.cos(ang).astype(f32), (8, 1))  # [128, 32]
    sinN = np.tile(np.sin(ang).astype(f32), (8, 1))

    ident = np.eye(128, dtype=f32)

    in_maps = []
    for c in range(N_CORES):
        wq_c = wqkvT[:, c * E_PER_CORE : (c + 1) * E_PER_CORE]
        wq_c = np.ascontiguousarray(
            wq_c.reshape(8, 128, E_PER_CORE).transpose(1, 0, 2)
        )
        in_maps.append(
            {
                "xT": xT,
                "wqT": wq_c,
                "woT": np.ascontiguousarray(
                    w_o[:, c * D_PER_CORE : (c + 1) * D_PER_CORE].T
                ).astype(bf16),
                "kv": kv[c].reshape(2 * B, 128, KVCOLS),
                "cosN": cosN,
                "sinN": sinN,
                "ident": ident,
            }
        )
    return in_maps


def _run(in_maps, trace=False, trace_kwargs=None):
    from concourse import bass_utils

    nc, in_names, out_name = _get_program()
    kwargs = {}
    if trace:
        kwargs["trace"] = True
        if trace_kwargs:
            kwargs.update(trace_kwargs)
    res = bass_utils.run_bass_kernel_spmd(
        nc, in_maps, core_ids=list(range(N_CORES)), **kwargs
    )
    return res


def kernel(x, w_qkv, w_o, cache_k, cache_v, cache_pos_k_rot, mask=None, **_ignored):
    """Full-input entry point: shards internally across 8 NeuronCores."""
    in_maps = _prep_inputs(x, w_qkv, w_o, cache_k, cache_v, cache_pos_k_rot)
    res = _run(in_maps)
    out = np.zeros((BS, DM), dtype=np.float32)
    for c in range(N_CORES):
        out += res.results[c]["out"].reshape(BS, DM)
    return out.reshape(B, SQ, DM)


# revision 38
# speedup vs baseline: 1.0955x; 1.0790x over previous
"""Trainium2 Bass kernel for nn_Attention_86431921864842.

Decode-style attention: B=16 batches, H=16 heads, Sq=16 new tokens,
4096-token KV cache, RoPE-extended 128-dim scores, fused QKV + output
projections.

Sharding: tensor-parallel over heads, 8 cores x 2 heads each.  Each core
receives the full x (bf16), its 2-head slice of w_qkv (transposed,
bf16), its 2-head column slice of w_o (transposed, bf16), and its heads'
K/rot/V caches as a single merged bf16 tensor per (head_local, batch):

  kv [32, 128, 6176] bf16 - cols 0:4096 = [cache_k; cache_pos_k_rot]^T
      (d2 = 128 on partitions, token on free); cols 4096:6176 = V cache
      tiled [p = token%128, tile, 64 dims + ones column] so the PV
      matmul also produces the softmax denominator.

The whole pipeline runs in single bf16 (inputs are f32; the 2e-2
correctness budget dwarfs bf16's ~1e-3 relative error), which halves
HBM traffic vs f32-grade storage and runs every matmul at the PE's
1 cycle/row bf16 rate.

Device per (b,h): S^T tiles via 1 bf16 matmul per 128-token tile
(k-chunk stationary, q [128,16] moving) -> exp (ACT, direct from PSUM,
bf16 out) -> PV accumulate (attn^T stationary, [V|1] moving) ->
per-query normalize -> o-proj partial.  PV for pair i is emitted after
S^T for pair i+1 (one-stage software pipeline) so the Tensor engine
never stalls on the exp.  Host sums the 8 partial o-proj outputs.
"""

import math
import os
import sys

import numpy as np

for _p in ("/opt/trn_rl_repo",):
    if _p not in sys.path and os.path.isdir(_p):
        sys.path.insert(0, _p)

B = 16
H = 16
SQ = 16
DM = 1024
DH = 64
SKV = 4096
ROPE_BASE = 10000.0
N_CORES = 8
H_PER_CORE = H // N_CORES  # 2
E_PER_CORE = H_PER_CORE * 3 * DH  # 384
D_PER_CORE = H_PER_CORE * DH  # 128
BS = B * SQ  # 256
N_KTILES = SKV // 128  # 32
VCOLS = N_KTILES * 65  # 2080
KVCOLS = SKV + VCOLS  # 6176
SCALE = 1.0 / math.sqrt(2 * DH)

_PROGRAM = None  # (nc, in_names, out_name)


def _build_program():
    import concourse.bass as bass
    import concourse.mybir as mybir
    import concourse.tile as tile
    from concourse import bacc

    f32 = mybir.dt.float32
    bf16 = mybir.dt.bfloat16
    Exp = mybir.ActivationFunctionType.Exp

    nc = bacc.Bacc(
        "TRN2",
        target_bir_lowering=False,
        debug=False,
        enable_asserts=False,
        num_devices=N_CORES,
    )

    x_d = nc.dram_tensor("xT", [128, 8, BS], bf16, kind="ExternalInput")
    wq_d = nc.dram_tensor("wqT", [128, 8, E_PER_CORE], bf16, kind="ExternalInput")
    wo_d = nc.dram_tensor("woT", [D_PER_CORE, DM], bf16, kind="ExternalInput")
    kv_d = nc.dram_tensor("kv", [2 * B, 128, KVCOLS], bf16, kind="ExternalInput")
    cos_d = nc.dram_tensor("cosN", [128, 32], f32, kind="ExternalInput")
    sin_d = nc.dram_tensor("sinN", [128, 32], f32, kind="ExternalInput")
    id_d = nc.dram_tensor("ident", [128, 128], f32, kind="ExternalInput")
    out_d = nc.dram_tensor("out", [2, 128, DM], bf16, kind="ExternalOutput")

    with tile.TileContext(nc) as tc:
        with (
            tc.tile_pool(name="const", bufs=1) as pc,
            tc.tile_pool(name="head", bufs=1) as ph,
            tc.tile_pool(name="rope", bufs=1) as pr,
            tc.tile_pool(name="kv", bufs=10) as pk,
            tc.tile_pool(name="exp", bufs=2) as pe,
            tc.tile_pool(name="small", bufs=2) as ps,
            tc.tile_pool(name="ps_s", bufs=2, space="PSUM") as pss,
            tc.tile_pool(name="ps_o", bufs=2, space="PSUM") as pso,
            tc.tile_pool(name="ps_m", bufs=2, space="PSUM") as psm,
        ):
            # ---- constants ----
            x_sb = pc.tile([128, 8, BS], bf16, tag="x")
            nc.sync.dma_start(x_sb[:], x_d[:])
            wq_sb = pc.tile([128, 8, E_PER_CORE], bf16, tag="wq")
            nc.sync.dma_start(wq_sb[:], wq_d[:])
            cos_sb = pc.tile([128, 32], f32, tag="cos")
            nc.sync.dma_start(cos_sb[:], cos_d[:])
            sin_sb = pc.tile([128, 32], f32, tag="sin")
            nc.sync.dma_start(sin_sb[:], sin_d[:])
            id_sb = pc.tile([128, 128], f32, tag="ident")
            nc.sync.dma_start(id_sb[:], id_d[:])
            wo_sb = pc.tile([128, DM], bf16, tag="wo")
            nc.sync.dma_start(wo_sb[:], wo_d[:])

            # ---- QKV projection: qkv_nat[bs, e_local] ----
            qkv_nat = ph.tile([128, 2, E_PER_CORE], f32, tag="qkv_nat")
            for j in range(2):
                psq = psm.tile([128, 512], f32, tag="misc", name=f"psq{j}")
                for dc in range(8):
                    nc.tensor.matmul(
                        psq[:, 0:E_PER_CORE],
                        lhsT=x_sb[:, dc, j * 128 : (j + 1) * 128],
                        rhs=wq_sb[:, dc, :],
                        start=(dc == 0),
                        stop=(dc == 7),
                    )
                nc.vector.tensor_copy(qkv_nat[:, j, :], psq[:, 0:E_PER_CORE])

            # ---- RoPE + transposes per local head ----
            cosb = cos_sb[:].unsqueeze(1).to_broadcast([128, 2, 32])
            sinb = sin_sb[:].unsqueeze(1).to_broadcast([128, 2, 32])
            q2T = []  # per head: [128, 256] f32 (d2, bs)
            q2B = []  # per head: [128, 256] bf16
            k2nT = []  # per head: [128, 256] f32
            k2nB = []  # per head: [128, 256] bf16
            vTh = []  # per head: [64, 256] f32 (dv, bs)
            for hl in range(2):
                base = hl * 3 * DH
                qs = qkv_nat[:, :, base : base + 64]
                ks = qkv_nat[:, :, base + 64 : base + 128]

                q2n = pr.tile([128, 2, 128], f32, tag="q2n")
                k2n = pr.tile([128, 2, 128], f32, tag="k2n")
                t1 = pr.tile([128, 2, 32], f32, tag="t1")
                t2 = pr.tile([128, 2, 32], f32, tag="t2")
                for src, dst in ((qs, q2n), (ks, k2n)):
                    x1 = src[:, :, 0:32]
                    x2 = src[:, :, 32:64]
                    nc.vector.tensor_copy(dst[:, :, 0:64], src)
                    nc.vector.tensor_mul(t1[:], x1, cosb)
                    nc.vector.tensor_mul(t2[:], x2, sinb)
                    nc.vector.tensor_sub(dst[:, :, 64:96], t1[:], t2[:])
                    nc.vector.tensor_mul(t1[:], x1, sinb)
                    nc.vector.tensor_mul(t2[:], x2, cosb)
                    nc.vector.tensor_add(dst[:, :, 96:128], t1[:], t2[:])

                q2T_h = ph.tile([128, BS], f32, tag=f"q2T_{hl}")
                k2nT_h = ph.tile([128, BS], f32, tag=f"k2nT_{hl}")
                vT_h = ph.tile([64, BS], f32, tag=f"vT_{hl}")
                for j in range(2):
                    pt = psm.tile([128, 512], f32, tag="misc")
                    nc.tensor.transpose(pt[:, 0:128], q2n[:, j, :], id_sb[:])
                    nc.vector.tensor_copy(q2T_h[:, j * 128 : (j + 1) * 128], pt[:, 0:128])
                    pt2 = psm.tile([128, 512], f32, tag="misc")
                    nc.tensor.transpose(pt2[:, 0:128], k2n[:, j, :], id_sb[:])
                    nc.vector.tensor_copy(
                        k2nT_h[:, j * 128 : (j + 1) * 128], pt2[:, 0:128]
                    )
                    pt3 = psm.tile([128, 512], f32, tag="misc")
                    nc.tensor.transpose(
                        pt3[0:64, 0:128],
                        qkv_nat[:, j, base + 128 : base + 192],
                        id_sb[:],
                    )
                    nc.vector.tensor_copy(vT_h[:, j * 128 : (j + 1) * 128], pt3[0:64, 0:128])

                q2b_h = ph.tile([128, BS], bf16, tag=f"q2b_{hl}")
                nc.vector.tensor_copy(q2b_h[:], q2T_h[:])
                k2nb_h = ph.tile([128, BS], bf16, tag=f"k2nb_{hl}")
                nc.vector.tensor_copy(k2nb_h[:], k2nT_h[:])

                q2T.append(q2T_h)
                q2B.append(q2b_h)
                k2nT.append(k2nT_h)
                k2nB.append(k2nb_h)
                vTh.append(vT_h)

            # ---- new-token V rows, pre-transposed to [s, (hl,b), 65] ----
            # (transposes are emitted inside the main loop, spread across
            # the first iterations, to keep the serial PE<->DVE ping-pong
            # off the startup critical path)
            vn_all = ph.tile([16, 2, B, 65], bf16, tag="vn_all")
            nc.vector.memset(vn_all[:, :, :, 64:65], 1.0)

            def emit_vn(hl, b):
                pvn = psm.tile([128, 512], f32, tag="misc")
                nc.tensor.transpose(
                    pvn[0:16, 0:64],
                    vTh[hl][:, b * 16 : (b + 1) * 16],
                    id_sb[0:64, 0:64],
                )
                nc.vector.tensor_copy(vn_all[:, hl, b, 0:64], pvn[0:16, 0:64])

            # val_sb[s, b, hl, dv] : normalized attention output (natural)
            val_sb = ph.tile([16, B, 2, 64], f32, tag="val_sb")

            # ---- main loop over (head_local, batch), PV pipelined 1 back ----
            def emit_pv(state):
                hl, b, expA, expB, kv_t = state
                ps_o = pso.tile([16, 65], f32, tag="o")
                for i in range(N_KTILES):
                    e = expA if i < 16 else expB
                    c = i if i < 16 else i - 16
                    nc.tensor.matmul(
                        ps_o[:],
                        lhsT=e[:, c * 16 : (c + 1) * 16],
                        rhs=kv_t[:, SKV + i * 65 : SKV + (i + 1) * 65],
                        start=(i == 0),
                        stop=False,
                    )
                nc.tensor.matmul(
                    ps_o[:],
                    lhsT=expB[0:16, 256:272],
                    rhs=vn_all[:, hl, b, :],
                    start=False,
                    stop=True,
                )
                rec = ps.tile([16, 1], f32, tag="rec")
                nc.vector.reciprocal(rec[:], ps_o[:, 64:65])
                nc.vector.tensor_mul(
                    val_sb[:, b, hl, :],
                    ps_o[:, 0:64],
                    rec[:, 0:1].to_broadcast([16, 64]),
                )

            # epilogue piece for one bs-chunk (8 batches x both heads)
            valT = ph.tile([128, 2, 128], bf16, tag="valT")
            out_sb = ph.tile([128, 2, DM], bf16, tag="out_sb")

            def emit_chunk_epilogue(j):
                pvt = psm.tile([128, 512], f32, tag="misc", name=f"pvt{j}")
                for bb in range(8):
                    b = j * 8 + bb
                    nc.tensor.transpose(
                        pvt[:, bb * 16 : (bb + 1) * 16],
                        val_sb[:, b, :, :],
                        id_sb[0:16, 0:16],
                    )
                nc.vector.tensor_copy(valT[:, j, :], pvt[:, 0:128])
                for h2 in range(2):
                    po = psm.tile([128, 512], f32, tag="misc", name=f"po{j}{h2}")
                    nc.tensor.matmul(
                        po[:],
                        lhsT=valT[:, j, :],
                        rhs=wo_sb[:, h2 * 512 : (h2 + 1) * 512],
                        start=True,
                        stop=True,
                    )
                    nc.vector.tensor_copy(
                        out_sb[:, j, h2 * 512 : (h2 + 1) * 512], po[:]
                    )
                # out goes on the gpsimd queue: the sync queue carries the kv
                # stream (a not-yet-ready out at the ring head would stall all
                # later kv transfers), and on the scalar queue the out's wait
                # head-of-line blocks the ACT sequencer, pushing every later
                # exp a full pair late and breaking the PV pipeline.
                nc.gpsimd.dma_start(out_d[j], out_sb[:, j, :])

            pending = None
            n_pv_done = 0
            for b in range(B):
                for hl in range(2):
                    bh = hl * B + b
                    kv_t = pk.tile([128, KVCOLS], bf16, tag="kv")
                    nc.sync.dma_start(kv_t[:], kv_d[bh])

                    if hl == 0:
                        # new-token V rows for this batch, both heads (needed
                        # by this pair's PV one iteration later)
                        emit_vn(0, b)
                        emit_vn(1, b)

                    # new-token scores (bf16, tiny; independent of the kv
                    # DMA, so it gives PE work at the bh boundary)
                    psn = psm.tile([16, 16], f32, tag="misc", name=f"psn{bh}")
                    nc.tensor.matmul(
                        psn[:],
                        lhsT=k2nB[hl][:, b * 16 : (b + 1) * 16],
                        rhs=q2B[hl][:, b * 16 : (b + 1) * 16],
                        start=True,
                        stop=True,
                    )

                    # S^T: one bf16 matmul per 128-token tile.  Scores and
                    # exp are split in half (separate PSUM + expT tiles) so
                    # the first exp is emitted - and its PE wait resolves -
                    # mid-scores; PV's first 16 chunks then depend only on
                    # expA, hiding the exp+sem latency inside the pair.
                    qb = q2B[hl][:, b * 16 : (b + 1) * 16]  # [128, 16] bf16
                    psA = pss.tile([128, 256], f32, tag="sTA")
                    psB = pss.tile([128, 256], f32, tag="sTB")
                    for i in range(16):
                        nc.tensor.matmul(
                            psA[:, i * 16 : (i + 1) * 16],
                            lhsT=kv_t[:, i * 128 : (i + 1) * 128],
                            rhs=qb,
                            start=True,
                            stop=True,
                        )
                    expA = pe.tile([128, 256], bf16, tag="expA")
                    nc.scalar.activation(expA[:], psA[:], Exp, scale=SCALE)
                    for i in range(16, N_KTILES):
                        nc.tensor.matmul(
                            psB[:, (i - 16) * 16 : (i - 15) * 16],
                            lhsT=kv_t[:, i * 128 : (i + 1) * 128],
                            rhs=qb,
                            start=True,
                            stop=True,
                        )
                    expB = pe.tile([128, 272], bf16, tag="expB")
                    nc.scalar.activation(expB[:, 0:256], psB[:], Exp, scale=SCALE)
                    nc.scalar.activation(
                        expB[0:16, 256:272], psn[:], Exp, scale=SCALE
                    )

                    if pending is not None:
                        emit_pv(pending)
                        n_pv_done += 1
                        if n_pv_done == 17:
                            # batches 0..7 (both heads) fully normalized:
                            # run the first output-chunk epilogue now
                            emit_chunk_epilogue(0)
                    pending = (hl, b, expA, expB, kv_t)
            emit_pv(pending)
            emit_chunk_epilogue(1)

    nc.compile()
    in_names = ["xT", "wqT", "woT", "kv", "cosN", "sinN", "ident"]
    return nc, in_names, "out"


def _get_program():
    global _PROGRAM
    if _PROGRAM is None:
        _PROGRAM = _build_program()
    return _PROGRAM


def _prep_inputs(x, w_qkv, w_o, cache_k, cache_v, cache_pos_k_rot):
    """Host-side sharding + layout prep. Returns list of per-core in_maps."""
    import ml_dtypes

    f32 = np.float32
    bf16 = ml_dtypes.bfloat16
    x = np.ascontiguousarray(x, dtype=f32)
    w_qkv = np.ascontiguousarray(w_qkv, dtype=f32)
    w_o = np.ascontiguousarray(w_o, dtype=f32)

    xT = np.ascontiguousarray(x.reshape(BS, DM).T).astype(bf16)
    # pre-tile to [p=128, dc=8, bs] so the const DMA is contiguous per row
    xT = np.ascontiguousarray(xT.reshape(8, 128, BS).transpose(1, 0, 2))

    wqkvT = np.ascontiguousarray(w_qkv.T).astype(bf16)  # [DM, 3*DM]

    # merged K2^T | V-tiles staging, per core: [2, B, 128, KVCOLS] bf16
    kv = np.empty((N_CORES, 2, B, 128, KVCOLS), dtype=bf16)
    kv[:, :, :, 0:64, 0:SKV] = cache_k.reshape(B, N_CORES, 2, SKV, DH).transpose(
        1, 2, 0, 4, 3
    )
    kv[:, :, :, 64:128, 0:SKV] = cache_pos_k_rot.reshape(
        B, N_CORES, 2, SKV, DH
    ).transpose(1, 2, 0, 4, 3)
    vpart = kv[:, :, :, :, SKV:].reshape(N_CORES, 2, B, 128, N_KTILES, 65)
    vpart[..., 0:64] = cache_v.reshape(B, N_CORES, 2, N_KTILES, 128, DH).transpose(
        1, 2, 0, 4, 3, 5
    )
    vpart[..., 64] = 1.0

    # RoPE tables, f32 math mirroring the reference
    j2 = np.arange(0, DH, 2, dtype=f32)
    inv_freq = (1.0 / (ROPE_BASE ** (j2 / f32(DH)))).astype(f32)
    pos = (SKV + np.arange(SQ)).astype(f32)
    ang = pos[:, None] * inv_freq[None, :]  # [16, 32]
    cosN = np.tile(np.cos(ang).astype(f32), (8, 1))  # [128, 32]
    sinN = np.tile(np.sin(ang).astype(f32), (8, 1))

    ident = np.eye(128, dtype=f32)

    in_maps = []
    for c in range(N_CORES):
        wq_c = wqkvT[:, c * E_PER_CORE : (c + 1) * E_PER_CORE]
        wq_c = np.ascontiguousarray(
            wq_c.reshape(8, 128, E_PER_CORE).transpose(1, 0, 2)
        )
        in_maps.append(
            {
                "xT": xT,
                "wqT": wq_c,
                "woT": np.ascontiguousarray(
                    w_o[:, c * D_PER_CORE : (c + 1) * D_PER_CORE].T
                ).astype(bf16),
                "kv": kv[c].reshape(2 * B, 128, KVCOLS),
                "cosN": cosN,
                "sinN": sinN,
                "ident": ident,
            }
        )
    return in_maps


def _run(in_maps, trace=False, trace_kwargs=None):
    from concourse import bass_utils

    nc, in_names, out_name = _get_program()
    kwargs = {}
    if trace:
        kwargs["trace"] = True
        if trace_kwargs:
            kwargs.update(trace_kwargs)
    res = bass_utils.run_bass_kernel_spmd(
        nc, in_maps, core_ids=list(range(N_CORES)), **kwargs
    )
    return res


def kernel(x, w_qkv, w_o, cache_k, cache_v, cache_pos_k_rot, mask=None, **_ignored):
    """Full-input entry point: shards internally across 8 NeuronCores."""
    in_maps = _prep_inputs(x, w_qkv, w_o, cache_k, cache_v, cache_pos_k_rot)
    res = _run(in_maps)
    out = np.zeros((BS, DM), dtype=np.float32)
    for c in range(N_CORES):
        out += res.results[c]["out"].reshape(BS, DM)
    return out.reshape(B, SQ, DM)
